# revision 1
# baseline (speedup 1.0000x reference)
"""DeepFM forward on 8 Trainium2 NeuronCores (Bass/Tile, SPMD).

Strategy: data-parallel over the batch (2048 rows/core), embedding tables
replicated. The first-order and second-order cat tables are fused host-side
into one [F_CAT*V, 65] table so a single indirect-DMA gather per batch tile
fetches both. The MLP runs in bf16 (fp32 accumulation in PSUM); batchnorm
statistics are exchanged with two tiny AllReduces. FM arithmetic stays fp32.

Layout: MLP operands are kept feature-major ("X.T": [feat, batch]) so the
contraction dim sits on SBUF partitions; gathered rows are transposed with
the DMA xbar (bf16). FM terms are computed in row layout during the gather
phase; the final logit is assembled in row layout via matmuls with the
hidden activations as the stationary operand.
"""

import numpy as np

# ---- problem constants (hardcoded per harness contract) ----
B, F_CAT, F_CONT, V, D = 16384, 26, 13, 100000, 64
H1, H2 = 1024, 512
N_CORES = 8
BN_EPS = 1e-5

CFG_FULL = dict(B=B, V=V, n_cores=N_CORES)

_P = 128


def _build_program(cfg):
    """Build the per-core SPMD Bass program. Returns (nc, names)."""
    import concourse.bacc as bacc
    import concourse.bass as bass
    import concourse.mybir as mybir
    import concourse.tile as tile
    from concourse.masks import make_identity

    F32, BF16, I32 = mybir.dt.float32, mybir.dt.float16, mybir.dt.int32
    AF = mybir.ActivationFunctionType
    OP = mybir.AluOpType
    AX = mybir.AxisListType
    P = _P

    ncore = cfg["n_cores"]
    Bfull = cfg["B"]
    Vv = cfg["V"]
    Bc = Bfull // ncore          # batch rows per core
    TB = Bc // P                 # batch tiles per core
    NB = min(512, Bc)            # matmul moving free dim
    NN = Bc // NB                # batch n-tiles
    TPN = NB // P                # 128-tiles per n-tile
    KC = F_CAT * D // P          # cat K-chunks (13)
    NKC = KC + 1                 # + cont chunk
    NM1 = H1 // P                # 8
    NM2 = H2 // P                # 4
    EW = D + 1                   # gathered row width (64 emb + 1 first-order)
    RW = F_CAT * EW              # gathered row bytes/4 per batch row (1690)
    rg = [list(range(ncore))]

    NQ = cfg.get("swdge_queues", 4)
    gqn = [0]
    nc = bacc.Bacc(num_devices=ncore, num_swdge_queues=NQ)

    idxg = nc.dram_tensor("idxg", [Bc, F_CAT], I32, kind="ExternalInput")
    cfin = nc.dram_tensor("cfin", [Bc, F_CONT], F32, kind="ExternalInput")
    bigt = nc.dram_tensor("bigt", [F_CAT * Vv, EW], F32, kind="ExternalInput")
    w1 = nc.dram_tensor("w1", [NKC * P, H1], BF16, kind="ExternalInput")
    w2 = nc.dram_tensor("w2", [H1, H2], BF16, kind="ExternalInput")
    w3 = nc.dram_tensor("w3", [P, NM2], BF16, kind="ExternalInput")
    ct2 = nc.dram_tensor("ct2", [F_CONT, D], F32, kind="ExternalInput")
    cmisc = nc.dram_tensor("cmisc", [P, 2 * F_CONT], F32, kind="ExternalInput")
    bnp = nc.dram_tensor("bnp", [P, 3 * NM1 + 3 * NM2 + 1], F32, kind="ExternalInput")
    out = nc.dram_tensor("out", [P, 2 * TB], F32, kind="ExternalOutput")

    with tile.TileContext(nc) as tc:
        with (
            tc.tile_pool(name="const", bufs=1) as cpool,
            tc.tile_pool(name="big", bufs=1) as bpool,
            tc.tile_pool(name="work", bufs=2) as wpool,
            tc.tile_pool(name="psmm", bufs=4, space="PSUM") as psmm,
            tc.tile_pool(name="pssm", bufs=4, space="PSUM") as pssm,
            tc.tile_pool(name="dram", bufs=1, space="DRAM") as dpool,
        ):
            # ---- constants ----
            w1sb = []
            for k in range(NKC):
                t = cpool.tile([P, H1], BF16, tag=f"w1_{k}")
                nc.sync.dma_start(out=t[:], in_=w1[k * P : (k + 1) * P, :])
                w1sb.append(t)
            w2sb = []
            for k in range(NM1):
                t = cpool.tile([P, H2], BF16, tag=f"w2_{k}")
                nc.sync.dma_start(out=t[:], in_=w2[k * P : (k + 1) * P, :])
                w2sb.append(t)
            w3sb = cpool.tile([P, NM2], BF16, tag="w3")
            nc.sync.dma_start(out=w3sb[:], in_=w3[:])
            ct2sb = cpool.tile([F_CONT, D], F32, tag="ct2")
            nc.sync.dma_start(out=ct2sb[:], in_=ct2[:])
            cmsb = cpool.tile([P, 2 * F_CONT], F32, tag="cmisc")
            nc.sync.dma_start(out=cmsb[:], in_=cmisc[:])
            bnsb = cpool.tile([P, 3 * NM1 + 3 * NM2 + 1], F32, tag="bnp")
            nc.sync.dma_start(out=bnsb[:], in_=bnp[:])
            ident = cpool.tile([P, P], F32, tag="ident")
            make_identity(nc, ident[:])
            eps_t = cpool.tile([P, 1], F32, tag="eps")
            nc.vector.memset(eps_t[:], BN_EPS)

            b1c = bnsb[:, 0:NM1]
            g1c = bnsb[:, NM1 : 2 * NM1]
            be1c = bnsb[:, 2 * NM1 : 3 * NM1]
            o2 = 3 * NM1
            b2c = bnsb[:, o2 : o2 + NM2]
            g2c = bnsb[:, o2 + NM2 : o2 + 2 * NM2]
            be2c = bnsb[:, o2 + 2 * NM2 : o2 + 3 * NM2]
            bias_col = bnsb[:, o2 + 3 * NM2 : o2 + 3 * NM2 + 1]
            t1b = cmsb[:, 0:F_CONT]
            rb = cmsb[:, F_CONT : 2 * F_CONT]

            # ---- persistent activations ----
            xtn = [
                bpool.tile([P, NKC, NB], BF16, tag=f"xtn_{n}", name=f"xtn_{n}")
                for n in range(NN)
            ]
            for n in range(NN):
                nc.vector.memset(xtn[n][:, KC, :], 0.0)
            h1t = [bpool.tile([P, Bc], BF16, tag=f"h1_{m}", name=f"h1_{m}") for m in range(NM1)]
            h2t = [bpool.tile([P, Bc], BF16, tag=f"h2_{m}", name=f"h2_{m}") for m in range(NM2)]

            # FM accumulators (col per batch tile)
            qcat = bpool.tile([P, TB], F32, tag="qcat")
            q2t = bpool.tile([P, TB], F32, tag="q2t")
            qct = bpool.tile([P, TB], F32, tag="qct")
            f1t = bpool.tile([P, TB], F32, tag="f1t")
            fct = bpool.tile([P, TB], F32, tag="fct")
            fm_all = bpool.tile([P, TB], F32, tag="fm")
            acc1 = bpool.tile([P, NM1 * NN], F32, tag="acc1")
            acc1s = bpool.tile([P, NM1 * NN], F32, tag="acc1s")
            acc2 = bpool.tile([P, NM2 * NN], F32, tag="acc2")
            acc2s = bpool.tile([P, NM2 * NN], F32, tag="acc2s")
            scr = bpool.tile([P, 64], F32, tag="scr")
            scrh = bpool.tile([P, 2048], BF16, tag="scrh")
            out_sb = bpool.tile([P, 2 * TB], F32, tag="outsb")

            # ---- phase A: gather + FM + transpose ----
            for t in range(TB):
                n, tp = t // TPN, t % TPN
                idx_t = wpool.tile([P, F_CAT], I32, tag="idx")
                nc.sync.dma_start(out=idx_t[:], in_=idxg[t * P : (t + 1) * P, :])
                rows = wpool.tile([P, RW], F32, tag="rows")
                for f in range(F_CAT):
                    inst = nc.gpsimd.indirect_dma_start(
                        out=rows[:, f * EW : (f + 1) * EW],
                        out_offset=None,
                        in_=bigt[:],
                        in_offset=bass.IndirectOffsetOnAxis(
                            ap=idx_t[:, f : f + 1], axis=0
                        ),
                    )
                    if NQ > 1:
                        inst.ins.queue = f"qPoolDynamic{(gqn[0] % NQ) or ''}"
                        gqn[0] += 1
                cf_t = wpool.tile([P, F_CONT], F32, tag="cf")
                nc.sync.dma_start(out=cf_t[:], in_=cfin[t * P : (t + 1) * P, :])

                rows_fe = rows[:].rearrange("p (f e) -> p f e", e=EW)
                cat3 = rows_fe[:, :, :D]            # [P, 26, 64]
                # cast cat cols to fp16 row-major (feeds transpose and q)
                xrow = wpool.tile([P, F_CAT * D], BF16, tag="xrow")
                nc.scalar.activation(
                    out=xrow[:].rearrange("p (f e) -> p f e", e=D),
                    in_=cat3, func=AF.Copy,
                )
                # q_cat = sum E^2 from the fp16 copy; scrh is a dummy output
                nc.scalar.activation(
                    out=scrh[:, : F_CAT * D], in_=xrow[:], func=AF.Square,
                    accum_out=qcat[:, t : t + 1],
                )
                # s = sum_f E  (keep d): [P, 64]
                s_t = wpool.tile([P, D], F32, tag="s")
                cat_df = rows[:].rearrange("p (f e) -> p e f", e=EW)[:, :D, :]
                nc.vector.tensor_reduce(
                    out=s_t[:], in_=cat_df, axis=AX.X, op=OP.add
                )
                # first-order cat: sum of col 64 of each block
                nc.vector.tensor_reduce(
                    out=f1t[:, t : t + 1],
                    in_=rows_fe[:, :, D : D + 1].rearrange("p f e -> p e f"),
                    axis=AX.X, op=OP.add,
                )
                # cont: transpose cf tile -> [13, P]
                tr_ps = pssm.tile([F_CONT, P], F32, tag="sm")
                nc.tensor.transpose(out=tr_ps[:], in_=cf_t[:], identity=ident[:])
                cfT = wpool.tile([F_CONT, P], F32, tag="cfT")
                nc.vector.tensor_copy(out=cfT[:], in_=tr_ps[:])
                # cont block of X.T (bf16) goes into the last K chunk
                nc.vector.tensor_copy(
                    out=xtn[n][0:F_CONT, KC, tp * P : (tp + 1) * P], in_=tr_ps[:]
                )
                # s_cont = cfT.T @ ct2 : [P, 64]
                ss_ps = pssm.tile([P, D], F32, tag="sm")
                nc.tensor.matmul(
                    out=ss_ps[:], lhsT=cfT[:], rhs=ct2sb[:], start=True, stop=True
                )
                nc.vector.tensor_tensor(
                    out=s_t[:], in0=s_t[:], in1=ss_ps[:], op=OP.add
                )
                # q2 = sum_d s^2
                nc.scalar.activation(
                    out=scr[:, :D], in_=s_t[:], func=AF.Square,
                    accum_out=q2t[:, t : t + 1],
                )
                # cont second-order: qc = sum_f cf^2 * r ; first-order fc
                c13a = wpool.tile([P, F_CONT], F32, tag="c13a")
                c13b = wpool.tile([P, F_CONT], F32, tag="c13b")
                nc.vector.tensor_tensor(out=c13a[:], in0=cf_t[:], in1=rb, op=OP.mult)
                nc.vector.tensor_tensor(out=c13b[:], in0=c13a[:], in1=cf_t[:], op=OP.mult)
                nc.vector.tensor_reduce(
                    out=qct[:, t : t + 1], in_=c13b[:], axis=AX.X, op=OP.add
                )
                nc.vector.tensor_tensor(out=c13a[:], in0=cf_t[:], in1=t1b, op=OP.mult)
                nc.vector.tensor_reduce(
                    out=fct[:, t : t + 1], in_=c13a[:], axis=AX.X, op=OP.add
                )
                # DMA-transpose (blocked 3D dest) into X.T chunks
                nc.sync.dma_start_transpose(
                    out=xtn[n][:, 0:KC, tp * P : (tp + 1) * P],
                    in_=xrow[:],
                )

            # fm = 0.5*(q2 - qcat - qc) + f1 + fc
            nc.vector.tensor_tensor(out=fm_all[:], in0=qcat[:], in1=qct[:], op=OP.add)
            nc.vector.tensor_tensor(out=fm_all[:], in0=q2t[:], in1=fm_all[:], op=OP.subtract)
            nc.vector.tensor_scalar(
                out=fm_all[:], in0=fm_all[:], scalar1=0.5, scalar2=None, op0=OP.mult
            )
            nc.vector.tensor_tensor(out=fm_all[:], in0=fm_all[:], in1=f1t[:], op=OP.add)
            nc.vector.tensor_tensor(out=fm_all[:], in0=fm_all[:], in1=fct[:], op=OP.add)

            # ---- phase B: layer 1 matmul ----
            for n in range(NN):
                for m in range(NM1):
                    ps = psmm.tile([P, NB], F32, tag="mm")
                    for k in range(NKC):
                        nc.tensor.matmul(
                            out=ps[:],
                            lhsT=w1sb[k][:, m * P : (m + 1) * P],
                            rhs=xtn[n][:, k, :],
                            start=(k == 0),
                            stop=(k == NKC - 1),
                        )
                    j = m * NN + n
                    nc.scalar.activation(
                        out=h1t[m][:, n * NB : (n + 1) * NB], in_=ps[:],
                        func=AF.Identity, bias=b1c[:, m : m + 1],
                        accum_out=acc1[:, j : j + 1],
                    )
                    nc.scalar.activation(
                        out=scrh[:, :NB], in_=h1t[m][:, n * NB : (n + 1) * NB],
                        func=AF.Square,
                        accum_out=acc1s[:, j : j + 1],
                    )

            # ---- phase C: BN1 (AllReduce stats) ----
            st1 = bpool.tile([P, 2 * NM1], F32, tag="st1")
            nc.vector.tensor_reduce(
                out=st1[:, :NM1],
                in_=acc1[:].rearrange("p (m n) -> p m n", n=NN),
                axis=AX.X, op=OP.add,
            )
            nc.vector.tensor_reduce(
                out=st1[:, NM1:],
                in_=acc1s[:].rearrange("p (m n) -> p m n", n=NN),
                axis=AX.X, op=OP.add,
            )
            st1i = dpool.tile([P, 2 * NM1], F32, tag="st1i")
            st1o = dpool.tile([P, 2 * NM1], F32, tag="st1o")
            nc.gpsimd.dma_start(out=st1i[:], in_=st1[:])
            nc.gpsimd.collective_compute(
                "AllReduce", OP.add, replica_groups=rg,
                ins=[st1i[:].opt()], outs=[st1o[:].opt()],
            )
            gst1 = bpool.tile([P, 2 * NM1], F32, tag="gst1")
            nc.gpsimd.dma_start(out=gst1[:], in_=st1o[:])

            mu1 = bpool.tile([P, NM1], F32, tag="mu1")
            var1 = bpool.tile([P, NM1], F32, tag="var1")
            a1 = bpool.tile([P, NM1], F32, tag="a1")
            bp1 = bpool.tile([P, NM1], F32, tag="bp1")
            inv_b = 1.0 / Bfull
            nc.vector.tensor_scalar(
                out=mu1[:], in0=gst1[:, :NM1], scalar1=inv_b, scalar2=None, op0=OP.mult
            )
            nc.vector.tensor_tensor(out=var1[:], in0=mu1[:], in1=mu1[:], op=OP.mult)
            nc.vector.tensor_scalar(
                out=a1[:], in0=gst1[:, NM1:], scalar1=inv_b, scalar2=None, op0=OP.mult
            )
            nc.vector.tensor_tensor(out=var1[:], in0=a1[:], in1=var1[:], op=OP.subtract)
            nc.scalar.activation(
                out=var1[:], in_=var1[:], func=AF.Sqrt, bias=eps_t[:, 0:1]
            )
            nc.vector.reciprocal(out=var1[:], in_=var1[:])
            nc.vector.tensor_tensor(out=a1[:], in0=g1c, in1=var1[:], op=OP.mult)
            nc.vector.tensor_tensor(out=bp1[:], in0=mu1[:], in1=a1[:], op=OP.mult)
            nc.vector.tensor_tensor(out=bp1[:], in0=be1c, in1=bp1[:], op=OP.subtract)
            for m in range(NM1):
                nc.scalar.activation(
                    out=h1t[m][:], in_=h1t[m][:], func=AF.Relu,
                    scale=a1[:, m : m + 1], bias=bp1[:, m : m + 1],
                )

            # ---- phase D: layer 2 ----
            for n in range(NN):
                for m in range(NM2):
                    ps = psmm.tile([P, NB], F32, tag="mm")
                    for k in range(NM1):
                        nc.tensor.matmul(
                            out=ps[:],
                            lhsT=w2sb[k][:, m * P : (m + 1) * P],
                            rhs=h1t[k][:, n * NB : (n + 1) * NB],
                            start=(k == 0),
                            stop=(k == NM1 - 1),
                        )
                    j = m * NN + n
                    nc.scalar.activation(
                        out=h2t[m][:, n * NB : (n + 1) * NB], in_=ps[:],
                        func=AF.Identity, bias=b2c[:, m : m + 1],
                        accum_out=acc2[:, j : j + 1],
                    )
                    nc.scalar.activation(
                        out=scrh[:, :NB], in_=h2t[m][:, n * NB : (n + 1) * NB],
                        func=AF.Square,
                        accum_out=acc2s[:, j : j + 1],
                    )

            # ---- phase E: BN2 ----
            st2 = bpool.tile([P, 2 * NM2], F32, tag="st2")
            nc.vector.tensor_reduce(
                out=st2[:, :NM2],
                in_=acc2[:].rearrange("p (m n) -> p m n", n=NN),
                axis=AX.X, op=OP.add,
            )
            nc.vector.tensor_reduce(
                out=st2[:, NM2:],
                in_=acc2s[:].rearrange("p (m n) -> p m n", n=NN),
                axis=AX.X, op=OP.add,
            )
            st2i = dpool.tile([P, 2 * NM2], F32, tag="st2i")
            st2o = dpool.tile([P, 2 * NM2], F32, tag="st2o")
            nc.gpsimd.dma_start(out=st2i[:], in_=st2[:])
            nc.gpsimd.collective_compute(
                "AllReduce", OP.add, replica_groups=rg,
                ins=[st2i[:].opt()], outs=[st2o[:].opt()],
            )
            gst2 = bpool.tile([P, 2 * NM2], F32, tag="gst2")
            nc.gpsimd.dma_start(out=gst2[:], in_=st2o[:])

            mu2 = bpool.tile([P, NM2], F32, tag="mu2")
            var2 = bpool.tile([P, NM2], F32, tag="var2")
            a2 = bpool.tile([P, NM2], F32, tag="a2")
            bp2 = bpool.tile([P, NM2], F32, tag="bp2")
            nc.vector.tensor_scalar(
                out=mu2[:], in0=gst2[:, :NM2], scalar1=inv_b, scalar2=None, op0=OP.mult
            )
            nc.vector.tensor_tensor(out=var2[:], in0=mu2[:], in1=mu2[:], op=OP.mult)
            nc.vector.tensor_scalar(
                out=a2[:], in0=gst2[:, NM2:], scalar1=inv_b, scalar2=None, op0=OP.mult
            )
            nc.vector.tensor_tensor(out=var2[:], in0=a2[:], in1=var2[:], op=OP.subtract)
            nc.scalar.activation(
                out=var2[:], in_=var2[:], func=AF.Sqrt, bias=eps_t[:, 0:1]
            )
            nc.vector.reciprocal(out=var2[:], in_=var2[:])
            nc.vector.tensor_tensor(out=a2[:], in0=g2c, in1=var2[:], op=OP.mult)
            nc.vector.tensor_tensor(out=bp2[:], in0=mu2[:], in1=a2[:], op=OP.mult)
            nc.vector.tensor_tensor(out=bp2[:], in0=be2c, in1=bp2[:], op=OP.subtract)
            for m in range(NM2):
                nc.scalar.activation(
                    out=h2t[m][:], in_=h2t[m][:], func=AF.Relu,
                    scale=a2[:, m : m + 1], bias=bp2[:, m : m + 1],
                )

            # ---- phase F: layer 3 + sigmoid + output ----
            for t in range(TB):
                psd = pssm.tile([P, 1], F32, tag="sm")
                for c in range(NM2):
                    nc.tensor.matmul(
                        out=psd[:],
                        lhsT=h2t[c][:, t * P : (t + 1) * P],
                        rhs=w3sb[:, c : c + 1],
                        start=(c == 0),
                        stop=(c == NM2 - 1),
                    )
                zt = wpool.tile([P, 1], F32, tag="zt")
                nc.vector.tensor_tensor(
                    out=zt[:], in0=fm_all[:, t : t + 1], in1=psd[:], op=OP.add
                )
                nc.scalar.activation(
                    out=out_sb[:, 2 * t + 1 : 2 * t + 2], in_=zt[:],
                    func=AF.Sigmoid, bias=bias_col,
                )
                nc.scalar.activation(
                    out=out_sb[:, 2 * t : 2 * t + 1],
                    in_=out_sb[:, 2 * t + 1 : 2 * t + 2],
                    func=AF.Copy, bias=1.0, scale=-1.0,
                )
            nc.sync.dma_start(out=out[:], in_=out_sb[:])

    return nc


def _prep_shared(inputs, cfg):
    """Host-side parameter prep (batch-independent). Returns dict of arrays
    shared by all cores."""
    import ml_dtypes

    Vv = cfg["V"]
    f32 = np.float32
    cat_t1 = np.asarray(inputs["cat_t1"], f32)          # [26, V]
    cat_t2 = np.asarray(inputs["cat_t2"], f32)          # [26, V, 64]
    cont_t1 = np.asarray(inputs["cont_t1"], f32)        # [13]
    cont_t2 = np.asarray(inputs["cont_t2"], f32)        # [13, 64]
    W1 = np.asarray(inputs["W1"], f32)                  # [2496, 1024]
    W2 = np.asarray(inputs["W2"], f32)
    W3 = np.asarray(inputs["W3"], f32)                  # [512, 1]
    b1 = np.asarray(inputs["b1"], f32)
    g1 = np.asarray(inputs["g1"], f32)
    be1 = np.asarray(inputs["be1"], f32)
    b2 = np.asarray(inputs["b2"], f32)
    g2 = np.asarray(inputs["g2"], f32)
    be2 = np.asarray(inputs["be2"], f32)
    b3 = np.asarray(inputs["b3"], f32)
    bias = np.asarray(inputs["bias"], f32)

    EW = D + 1
    bigt = np.empty((F_CAT * Vv, EW), f32)
    bigt[:, :D] = cat_t2.reshape(F_CAT * Vv, D)
    bigt[:, D] = cat_t1.reshape(F_CAT * Vv)

    ncat = F_CAT * D  # 1664
    NKC = ncat // _P + 1
    W1eff = np.einsum("fd,fdh->fh", cont_t2, W1[ncat:].reshape(F_CONT, D, H1))
    # cont rows (folded through cont_t2) live at the start of the last K chunk
    w1p = np.zeros((NKC * _P, H1), f32)
    w1p[:ncat] = W1[:ncat]
    w1p[ncat : ncat + F_CONT] = W1eff
    bf16 = np.float16

    NM1, NM2 = H1 // _P, H2 // _P
    bnp = np.zeros((_P, 3 * NM1 + 3 * NM2 + 1), f32)
    bnp[:, 0:NM1] = b1.reshape(NM1, _P).T
    bnp[:, NM1 : 2 * NM1] = g1.reshape(NM1, _P).T
    bnp[:, 2 * NM1 : 3 * NM1] = be1.reshape(NM1, _P).T
    o2 = 3 * NM1
    bnp[:, o2 : o2 + NM2] = b2.reshape(NM2, _P).T
    bnp[:, o2 + NM2 : o2 + 2 * NM2] = g2.reshape(NM2, _P).T
    bnp[:, o2 + 2 * NM2 : o2 + 3 * NM2] = be2.reshape(NM2, _P).T
    bnp[:, o2 + 3 * NM2] = float(bias[0]) + float(b3[0])

    cmisc = np.zeros((_P, 2 * F_CONT), f32)
    cmisc[:, :F_CONT] = cont_t1[None, :]
    cmisc[:, F_CONT:] = (cont_t2**2).sum(axis=1)[None, :]

    return {
        "bigt": bigt,
        "w1": w1p.astype(bf16),
        "w2": W2.astype(bf16),
        "w3": W3[:, 0].reshape(NM2, _P).T.astype(bf16).copy(),
        "ct2": cont_t2,
        "cmisc": cmisc,
        "bnp": bnp,
    }


def _prep_in_maps(inputs, cfg):
    """Build the per-core input maps (shard batch, replicate params)."""
    ncore = cfg["n_cores"]
    Vv = cfg["V"]
    Bc = cfg["B"] // ncore
    shared = _prep_shared(inputs, cfg)
    cat = np.asarray(inputs["cat_feats"]).astype(np.int32)
    cont = np.asarray(inputs["cont_feats"], np.float32)
    idxg = cat + (np.arange(F_CAT, dtype=np.int32) * Vv)[None, :]
    in_maps = []
    for c in range(ncore):
        m = dict(shared)
        m["idxg"] = idxg[c * Bc : (c + 1) * Bc]
        m["cfin"] = cont[c * Bc : (c + 1) * Bc]
        in_maps.append(m)
    return in_maps


def _unshard(results, cfg):
    ncore = cfg["n_cores"]
    Bc = cfg["B"] // ncore
    TB = Bc // _P
    outs = []
    for c in range(ncore):
        a = results[c]["out"]  # [128, 2*TB]
        outs.append(a.reshape(_P, TB, 2).transpose(1, 0, 2).reshape(Bc, 2))
    return np.concatenate(outs, axis=0)


_CACHE = {}


def _get_program(cfg_key):
    if cfg_key not in _CACHE:
        cfg = dict(B=cfg_key[0], V=cfg_key[1], n_cores=cfg_key[2])
        nc = _build_program(cfg)
        nc.finalize()
        _CACHE[cfg_key] = nc
    return _CACHE[cfg_key]


def run(inputs, trace=False, cfg=None):
    from concourse import bass_utils

    cfg = cfg or CFG_FULL
    nc = _get_program((cfg["B"], cfg["V"], cfg["n_cores"]))
    in_maps = _prep_in_maps(inputs, cfg)
    res = bass_utils.run_bass_kernel_spmd(
        nc, in_maps, core_ids=list(range(cfg["n_cores"])), trace=trace
    )
    return _unshard(res.results, cfg), res


def kernel(**inputs) -> np.ndarray:
    out, _ = run(inputs, trace=False)
    return out

